# revision 1
# baseline (speedup 1.0000x reference)
"""BitSwarmLinear Trainium2 kernel.

Computation (reference):
    swarm_sum = population.sum(axis=2)          # (out, in)
    w_eff     = sign(swarm_sum), sign(0) -> +1  # (out, in), +-1
    y         = einsum("bsi,oi->bso", x, w_eff) # (4, 4096, out)

Distribution (8 NeuronCores, tensor-parallel on out_features):
    - population sharded on out_features: each core gets its 256 rows,
      reduces + binarizes them and computes its 256 output columns.
    - x replicated to every core, staged pre-transposed/tiled as bf16 so the
      contraction dim lands on SBUF partitions with fully-contiguous DMA.
    - outputs gathered on the host along the feature dim.

Host staging (lossless / layout-only):
    - population is exactly +-1.0 -> int8, rearranged swarm-major
      [32, out_c, in]: cuts the dominant input stream 4x and lets the DMA
      engines' inline CCE ALU do the swarm reduction during transfer.
    - x -> bf16 x^T, tiled [tb, 128 ki, 16 ko, TB tok] so every DMA line is
      a 32KB contiguous run (line-rate HBM).
    - y comes back bf16 tile-major; host restores [b, s, out] f32.

Per-core device pipeline:
    1. Four parallel SWDGE accumulate chains (8 DMAs each, CCE int8 add)
       reduce the swarm axis while transferring; DVE merges 4 partials,
       binarizes via (s >= 0) * 2 - 1 (exact: sums are even ints, 0 -> +1).
    2. PE-transpose the sign matrix into W [in(part), out] bf16 (SBUF
       resident, 1 MB).
    3. Stream x^T tiles (4MB contiguous DMAs, deep prefetch); per 128-token
       block accumulate 16 K-tile matmuls into PSUM [128 tok, 256 out]
       (fp32), round to bf16, store on the scalar HWDGE ring.
"""

import os
import sys

import numpy as np

for _p in ("/root/.axon_site/_ro/trn_rl_repo", "/opt/trn_rl_repo"):
    if os.path.isdir(_p) and _p not in sys.path:
        sys.path.append(_p)

import ml_dtypes

# bass_utils' axon trace path imports antenv.axon_hooks, which this image
# lacks. Provide it (backed by the ctypes NTFF hook) so running with
# BASS_TRACE=1 works instead of crashing on the import.
try:
    import antenv.axon_hooks  # noqa: F401
except ImportError:
    try:
        import types as _types

        from trn_agent_boot.trn_boot import _ntff_profile_via_ctypes

        _hooks = _types.ModuleType("antenv.axon_hooks")
        _ntff_hook = _ntff_profile_via_ctypes("/opt/axon/libaxon_pjrt.so")
        _hooks.get_axon_ntff_profile_hook = lambda: _ntff_hook
        _hooks.set_axon_ntff_profile_hook = lambda h: None
        sys.modules["antenv.axon_hooks"] = _hooks
    except Exception:
        pass

import concourse.bass as bass  # noqa: F401  (AP helpers)
import concourse.mybir as mybir
import concourse.tile as tile
from concourse import bacc
from concourse.bass_utils import run_bass_kernel_spmd
from concourse.masks import make_identity

P = 128
IN_F = 2048
SWARM = 32
OUT_F = 2048
N_CORES = 8
OUT_C = OUT_F // N_CORES  # 256 out features per core
TOKENS = 4 * 4096

F32 = mybir.dt.float32
BF16 = mybir.dt.bfloat16
U8 = mybir.dt.uint8
I16 = mybir.dt.int16

# token-block per x^T DMA / output store
TB = 1024
# x^T prefetch depth (SBUF: 32KB/partition each at TB=1024)
XT_BUFS = 4
# swarm-slice staging depth for the reduction
POP_BUFS = 4


def build_nc(tokens: int = TOKENS, out_c: int = OUT_C, in_f: int = IN_F,
             reps: int = 1):
    """Build the per-core Bass program (same program on all 8 cores).

    reps>1 repeats the whole pipeline back-to-back (timing harness only)."""
    ko_tiles = in_f // P          # 16 K-tiles
    oc_groups = out_c // P        # 2 groups of 128 out rows
    tb_count = tokens // TB
    m_per_tb = TB // P

    nc = bacc.Bacc(
        "TRN2",
        target_bir_lowering=False,
        debug=False,
        enable_asserts=False,
        num_devices=N_CORES,
    )

    xT = nc.dram_tensor("xT", [tb_count, P, ko_tiles, TB], BF16,
                        kind="ExternalInput")
    pop = nc.dram_tensor("pop", [SWARM, out_c, in_f], U8,
                         kind="ExternalInput")
    y = nc.dram_tensor("y", [tb_count, P, m_per_tb, out_c], BF16,
                       kind="ExternalOutput")

    xr = xT.ap()                                              # [tb,128,ko,TB]
    pr = pop.ap().rearrange("s (g p) i -> s p g i", p=P)      # [32,128,oc,in]
    yr = y.ap()                                               # [tb,128,m,oc*P]

    with tile.TileContext(nc) as tc:
        with (
            tc.tile_pool(name="const", bufs=1) as const_pool,
            tc.tile_pool(name="pops", bufs=POP_BUFS) as pop_pool,
            tc.tile_pool(name="acc", bufs=1) as acc_pool,
            tc.tile_pool(name="sgn", bufs=oc_groups) as sgn_pool,
            tc.tile_pool(name="wsb", bufs=1) as w_pool,
            tc.tile_pool(name="xt", bufs=XT_BUFS) as x_pool,
            tc.tile_pool(name="ystage", bufs=2) as y_pool,
            tc.tile_pool(name="psum_t", bufs=2, space="PSUM") as psum_t_pool,
            tc.tile_pool(name="psum_y", bufs=4, space="PSUM") as psum_y_pool,
        ):
            ident = const_pool.tile([P, P], F32)
            make_identity(nc, ident[:])

            for _rep in range(reps):
                _emit_body(
                    nc, ident, w_pool, pop_pool, acc_pool, sgn_pool, x_pool,
                    y_pool, psum_t_pool, psum_y_pool, pr, xr, yr,
                    oc_groups, ko_tiles, tb_count, m_per_tb, out_c, in_f,
                )

    nc.compile()  # bacc register allocation / DCE — required before codegen
    return nc


def _emit_body(nc, ident, w_pool, pop_pool, acc_pool, sgn_pool, x_pool,
               y_pool, psum_t_pool, psum_y_pool, pr, xr, yr,
               oc_groups, ko_tiles, tb_count, m_per_tb, out_c, in_f):
    # W in [in(part), ko, out] bf16 — matmul rhs tiles, SBUF-resident
    w_sb = w_pool.tile([P, ko_tiles, out_c], BF16, tag="wsb")

    # ---- Stage 1: swarm reduction as packed-byte adds.
    # pop is staged {0,1} uint8; 2 bytes are summed per int16 lane — no
    # carries cross byte lanes (every lane stays in [0, 32]), and int16
    # values <= 8224 survive the DVE's fp32 ALU exactly.
    acc = acc_pool.tile([P, oc_groups, in_f // 2], I16, tag="acc")
    for s in range(SWARM):
        pt = pop_pool.tile([P, oc_groups, in_f], U8, tag="pops")
        eng = nc.sync if s % 2 == 0 else nc.scalar
        eng.dma_start(pt[:], pr[s])
        if s == 0:
            nc.vector.tensor_copy(out=acc[:], in_=pt[:].bitcast(I16))
        else:
            nc.vector.tensor_add(acc[:], acc[:], pt[:].bitcast(I16))

    # ---- Stage 2: binarize + PE-transpose into W [in, out] bf16
    acc_u8 = acc[:].bitcast(U8)  # [128, oc, in] counts in [0, 32]
    for oc in range(oc_groups):
        sgn = sgn_pool.tile([P, in_f], F32, tag="sgn", name=f"sgn{oc}")
        # count >= 16  <=>  swarm_sum >= 0; w = (count >= 16) * 2 - 1
        nc.vector.tensor_scalar(
            out=sgn[:], in0=acc_u8[:, oc, :], scalar1=16, scalar2=2.0,
            op0=mybir.AluOpType.is_ge, op1=mybir.AluOpType.mult,
        )
        nc.vector.tensor_scalar(
            out=sgn[:], in0=sgn[:], scalar1=1.0, scalar2=None,
            op0=mybir.AluOpType.subtract,
        )
        for k in range(ko_tiles):
            pt_ps = psum_t_pool.tile([P, P], F32, tag="tps")
            nc.tensor.transpose(
                pt_ps[:], sgn[:, k * P : (k + 1) * P], ident[:]
            )
            nc.vector.tensor_copy(
                out=w_sb[:, k, oc * P : (oc + 1) * P], in_=pt_ps[:]
            )

    # ---- Stage 3: stream x^T, matmul, store y (bf16)
    for tb in range(tb_count):
        xt = x_pool.tile([P, ko_tiles, TB], BF16, tag="xt")
        nc.sync.dma_start(xt[:], xr[tb])
        ystage = y_pool.tile([P, m_per_tb, out_c], BF16, tag="ys")
        for m in range(m_per_tb):
            ps = psum_y_pool.tile([P, out_c], F32, tag="yps")
            for k in range(ko_tiles):
                nc.tensor.matmul(
                    ps[:],
                    xt[:, k, m * P : (m + 1) * P],
                    w_sb[:, k, :],
                    start=(k == 0),
                    stop=(k == ko_tiles - 1),
                )
            nc.vector.tensor_copy(out=ystage[:, m, :], in_=ps[:])
        # stores ride the ACT HWDGE ring; loads own the SP ring
        nc.scalar.dma_start(yr[tb], ystage[:])


_NC_CACHE: dict = {}


def _get_nc(tokens=TOKENS, out_c=OUT_C, in_f=IN_F):
    key = (tokens, out_c, in_f)
    if key not in _NC_CACHE:
        _NC_CACHE[key] = build_nc(*key)
    return _NC_CACHE[key]


def stage_x(x: np.ndarray, tokens: int, in_f: int):
    """x [b, s, in] f32 -> tiled bf16 [tb, 128 ki, ko, TB] of x^T."""
    xb = np.ascontiguousarray(
        x.reshape(tokens, in_f).T
    ).astype(ml_dtypes.bfloat16)  # [in, tokens]
    ko = in_f // P
    tb = tokens // TB
    # (ko ki) (tb t) -> tb ki ko t
    return np.ascontiguousarray(
        xb.reshape(ko, P, tb, TB).transpose(2, 1, 0, 3)
    )


def stage_pop_slice(pop_c: np.ndarray):
    """pop slice [out_c, in, 32] (+-1.0 f32) -> swarm-major {0,1} uint8
    [32, out_c, in]. Lossless recode: -1 -> 0, +1 -> 1."""
    return np.ascontiguousarray(
        (pop_c > 0).astype(np.uint8).transpose(2, 0, 1)
    )


def unstage_y(y_dev: np.ndarray, tokens: int, out_c: int):
    """y [tb, 128 p, m, out_c] bf16 -> [tokens, out_c] f32
    (token = tb*TB + m*128 + p)."""
    return (
        y_dev.astype(np.float32)
        .transpose(0, 2, 1, 3)
        .reshape(tokens, out_c)
    )


def prep_inputs(x: np.ndarray, population: np.ndarray):
    tokens = x.shape[0] * x.shape[1]
    in_f = x.shape[2]
    xT = stage_x(x, tokens, in_f)
    out_c = population.shape[0] // N_CORES
    in_maps = []
    for c in range(N_CORES):
        pop_c = stage_pop_slice(population[c * out_c : (c + 1) * out_c])
        in_maps.append({"xT": xT, "pop": pop_c})
    return in_maps, tokens, out_c, in_f


def kernel(x: np.ndarray, population: np.ndarray):
    in_maps, tokens, out_c, in_f = prep_inputs(x, population)
    nc = _get_nc(tokens, out_c, in_f)
    res = run_bass_kernel_spmd(nc, in_maps, core_ids=list(range(N_CORES)))
    y_full = np.concatenate(
        [unstage_y(r["y"], tokens, out_c) for r in res.results], axis=1
    )
    return y_full.reshape(x.shape[0], x.shape[1], population.shape[0])



# revision 3
# speedup vs baseline: 1.2899x; 1.2899x over previous
"""BitSwarmLinear Trainium2 kernel.

Computation (reference):
    swarm_sum = population.sum(axis=2)          # (out, in)
    w_eff     = sign(swarm_sum), sign(0) -> +1  # (out, in), +-1
    y         = einsum("bsi,oi->bso", x, w_eff) # (4, 4096, out)

Distribution (8 NeuronCores, 2D: 2-way tokens x 4-way out_features):
    core c -> token half c//4 (8192 tokens), out quarter c%4 (512 cols).
    Per-core DMA drops to ~52 MiB (x 32 + pop 16 + y 8) = ~150us at
    358 GB/s, under the 128x128 PE's bf16 floor of ~220us for the
    8.6 G-MAC/core matmul -> compute-bound.

Host staging (lossless / layout-only):
    - population is exactly +-1.0 -> one BIT per element, two swarm
      members per byte (lo/hi nibble), laid out pre-transposed
      [og, in_part, pair, ko, out] so the device reduction lands
      directly in the matmul-lhsT layout W^T[in, out]. 8x less DMA
      than the u8 {0,1} recode and zero on-device transposes.
    - x -> bf16 x^T, tiled [tb, 128 ki, 16 ko, TB tok]: every DMA line
      is a contiguous 32 KB run (line-rate HBM).
    - y returns bf16 [tb, out_part, og, tok] tile-major; host restores
      [b, s, out] f32.

Per-core device pipeline:
    1. W-prep per og block (128 out cols): DMA 4x 1MB nibble chunks
       (ACT ring); DVE sums 16 byte-planes in two halves (nibble
       counts <= 8 each, uint16-packed adds, exact in the fp32 ALU),
       unpacks via shift/mask, binarizes (count >= 16 <=> sum >= 0,
       0 -> +1) straight into W [128 in_part, ko, 512 out] bf16.
    2. Stream x^T tiles (4 MB contiguous DMAs on the SP ring, 3-deep);
       per 1024-token tile run 4 og x 2 th PSUM groups of 16
       accumulating matmuls (W slice stationary, x moving, N=512);
       DVE rounds PSUM to bf16; 1 MB stores ride the ACT ring.
"""

import os
import sys

import numpy as np

for _p in ("/root/.axon_site/_ro/trn_rl_repo", "/opt/trn_rl_repo"):
    if os.path.isdir(_p) and _p not in sys.path:
        sys.path.append(_p)

import ml_dtypes

# bass_utils' axon trace path imports antenv.axon_hooks, which this image
# lacks. Provide it (backed by the ctypes NTFF hook) so running with
# BASS_TRACE=1 works instead of crashing on the import.
try:
    import antenv.axon_hooks  # noqa: F401
except ImportError:
    try:
        import types as _types

        from trn_agent_boot.trn_boot import _ntff_profile_via_ctypes

        _hooks = _types.ModuleType("antenv.axon_hooks")
        _ntff_hook = _ntff_profile_via_ctypes("/opt/axon/libaxon_pjrt.so")
        _hooks.get_axon_ntff_profile_hook = lambda: _ntff_hook
        _hooks.set_axon_ntff_profile_hook = lambda h: None
        sys.modules["antenv.axon_hooks"] = _hooks
    except Exception:
        pass

import concourse.bass as bass  # noqa: F401  (AP helpers)
import concourse.mybir as mybir
import concourse.tile as tile
from concourse import bacc
from concourse.bass_utils import run_bass_kernel_spmd

P = 128
IN_F = 2048
SWARM = 32
OUT_F = 2048
N_CORES = 8
TOK_WAYS = 2
OUT_WAYS = 4
TOKENS = 4 * 4096
TOK_C = TOKENS // TOK_WAYS      # 8192 tokens per core
OUT_C = OUT_F // OUT_WAYS       # 512 out features per core
KO = IN_F // P                  # 16 K-tiles
OG = OUT_C // P                 # 4 out groups of 128
PAIRS = SWARM // 2              # 16 byte-planes (2 swarm bits per byte)
TB = 1024                       # tokens per x tile / y store
TH = TB // 512                  # 2 PSUM groups of 512 tokens per tile

F32 = mybir.dt.float32
BF16 = mybir.dt.bfloat16
U8 = mybir.dt.uint8
U16 = mybir.dt.uint16

LO_MASK = 0x0F0F  # low nibble of both bytes in an int16 lane

XT_BUFS = 3
PT_BUFS = 3


def build_nc(tb_count: int = TOK_C // TB):
    """Build the per-core Bass program (same program on all 8 cores)."""
    nc = bacc.Bacc(
        "TRN2",
        target_bir_lowering=False,
        debug=False,
        enable_asserts=False,
        num_devices=N_CORES,
    )

    xT = nc.dram_tensor("xT", [tb_count, P, KO, TB], BF16,
                        kind="ExternalInput")
    # nibble-packed population: [og, in_part, pair, ko, out128]
    nib = nc.dram_tensor("nib", [OG, P, PAIRS, KO, P], U8,
                         kind="ExternalInput")
    y = nc.dram_tensor("y", [tb_count, P, OG, TB], BF16,
                       kind="ExternalOutput")

    xr = xT.ap()
    nr = nib.ap()
    yr = y.ap()

    with tile.TileContext(nc) as tc:
        with (
            tc.tile_pool(name="wsb", bufs=1) as w_pool,
            tc.tile_pool(name="pt", bufs=PT_BUFS) as pt_pool,
            tc.tile_pool(name="acc", bufs=2) as acc_pool,
            tc.tile_pool(name="tmp", bufs=2) as tmp_pool,
            tc.tile_pool(name="xt", bufs=XT_BUFS) as x_pool,
            tc.tile_pool(name="ys", bufs=2) as y_pool,
            tc.tile_pool(name="psum_y", bufs=4, space="PSUM") as psum_pool,
        ):
            # W^T [in_part, ko, out] bf16 -- matmul lhsT slices, SBUF-resident
            w_sb = w_pool.tile([P, KO, OUT_C], BF16, tag="wsb")

            # ---- Stage 1: swarm reduction + binarize, one og block at a time
            for og in range(OG):
                # two half-accumulators: 8 byte-planes each keeps every
                # nibble count <= 8 (no carry across nibbles)
                acc = acc_pool.tile([P, 2, KO, P // 2], U16, tag="acc")
                for pc in range(4):  # 4 chunks x 4 pair-planes (1 MB each)
                    pt = pt_pool.tile([P, 4, KO, P], U8, tag="pt")
                    nc.scalar.dma_start(pt[:], nr[og, :, 4 * pc : 4 * pc + 4])
                    pt16 = pt[:].bitcast(U16)  # [128, 4, KO, 64]
                    dst = acc[:, pc // 2]
                    if pc % 2 == 0:
                        nc.vector.tensor_add(dst, pt16[:, 0], pt16[:, 1])
                    else:
                        nc.vector.tensor_add(dst, dst, pt16[:, 0])
                        nc.vector.tensor_add(dst, dst, pt16[:, 1])
                    nc.vector.tensor_add(dst, dst, pt16[:, 2])
                    nc.vector.tensor_add(dst, dst, pt16[:, 3])
                # cnt16 = (a&LO) + ((a>>4)&LO) + (b&LO) + ((b>>4)&LO)
                # (byte lanes of cnt16 = per-element bit counts in [0, 32])
                a16, b16 = acc[:, 0], acc[:, 1]
                t0 = tmp_pool.tile([P, KO, P // 2], U16, tag="t0")
                u0 = tmp_pool.tile([P, KO, P // 2], U16, tag="u0")
                nc.vector.tensor_scalar(
                    out=t0[:], in0=a16, scalar1=LO_MASK, scalar2=None,
                    op0=mybir.AluOpType.bitwise_and,
                )
                nc.vector.tensor_scalar(
                    out=u0[:], in0=a16, scalar1=4, scalar2=LO_MASK,
                    op0=mybir.AluOpType.logical_shift_right,
                    op1=mybir.AluOpType.bitwise_and,
                )
                nc.vector.tensor_add(t0[:], t0[:], u0[:])
                nc.vector.tensor_scalar(
                    out=u0[:], in0=b16, scalar1=LO_MASK, scalar2=None,
                    op0=mybir.AluOpType.bitwise_and,
                )
                nc.vector.tensor_add(t0[:], t0[:], u0[:])
                nc.vector.tensor_scalar(
                    out=u0[:], in0=b16, scalar1=4, scalar2=LO_MASK,
                    op0=mybir.AluOpType.logical_shift_right,
                    op1=mybir.AluOpType.bitwise_and,
                )
                nc.vector.tensor_add(t0[:], t0[:], u0[:])
                cnt8 = t0[:].bitcast(U8)  # [128, KO, 128] counts in [0, 32]
                wslice = w_sb[:, :, og * P : (og + 1) * P]
                # count >= 16  <=>  swarm_sum >= 0; w = (count >= 16)*2 - 1
                nc.vector.tensor_scalar(
                    out=wslice, in0=cnt8, scalar1=16, scalar2=2.0,
                    op0=mybir.AluOpType.is_ge, op1=mybir.AluOpType.mult,
                )
                nc.vector.tensor_scalar(
                    out=wslice, in0=wslice, scalar1=1.0, scalar2=None,
                    op0=mybir.AluOpType.subtract,
                )

            # ---- Stage 2: stream x^T, matmul (W stationary), store y
            for tb in range(tb_count):
                xt = x_pool.tile([P, KO, TB], BF16, tag="xt")
                nc.sync.dma_start(xt[:], xr[tb])
                ystage = y_pool.tile([P, OG, TB], BF16, tag="ys")
                for og in range(OG):
                    for th in range(TH):
                        ps = psum_pool.tile([P, 512], F32, tag="yps")
                        for k in range(KO):
                            nc.tensor.matmul(
                                ps[:],
                                w_sb[:, k, og * P : (og + 1) * P],
                                xt[:, k, th * 512 : (th + 1) * 512],
                                start=(k == 0),
                                stop=(k == KO - 1),
                            )
                        nc.vector.tensor_copy(
                            out=ystage[:, og, th * 512 : (th + 1) * 512],
                            in_=ps[:],
                        )
                nc.scalar.dma_start(yr[tb], ystage[:])

    nc.compile()
    return nc


_NC_CACHE: dict = {}


def _get_nc(tb_count: int = TOK_C // TB):
    if tb_count not in _NC_CACHE:
        _NC_CACHE[tb_count] = build_nc(tb_count)
    return _NC_CACHE[tb_count]


def stage_x(x: np.ndarray):
    """x [b, s, in] f32 -> tiled bf16 x^T [tb_total, 128 ki, ko, TB]."""
    tokens = x.shape[0] * x.shape[1]
    xb = np.ascontiguousarray(
        x.reshape(tokens, IN_F).T
    ).astype(ml_dtypes.bfloat16)  # [in, tokens]
    tbt = tokens // TB
    # (ko ki) (tb t) -> tb ki ko t
    return np.ascontiguousarray(
        xb.reshape(KO, P, tbt, TB).transpose(2, 1, 0, 3)
    )


def stage_pop_quarter(pop_q: np.ndarray):
    """pop slice [512 out, in, 32] (+-1.0 f32) -> nibble-packed u8
    [og, ki, pair, ko, out128]; byte = bit(s=p) | bit(s=16+p) << 4.
    Lossless layout-only recode (one bit per population element)."""
    b = (pop_q > 0).astype(np.uint8)  # [out 512, in 2048, s 32]
    b = b.reshape(OG, P, KO, P, 2, PAIRS)  # [og, o, ko, ki, two, p]
    nib = b[..., 0, :] | (b[..., 1, :] << 4)  # [og, o, ko, ki, p]
    return np.ascontiguousarray(nib.transpose(0, 3, 4, 2, 1))


def unstage_y(y_dev: np.ndarray):
    """y [tb, 128 r, og, TB t] bf16 -> [tok_c, out_c] f32
    (token = tb*TB + t, out = og*128 + r)."""
    tbc = y_dev.shape[0]
    return (
        y_dev.astype(np.float32)
        .transpose(0, 3, 2, 1)
        .reshape(tbc * TB, OUT_C)
    )


def prep_inputs(x: np.ndarray, population: np.ndarray):
    xT = stage_x(x)
    tb_half = TOK_C // TB
    nib_q = [
        stage_pop_quarter(population[q * OUT_C : (q + 1) * OUT_C])
        for q in range(OUT_WAYS)
    ]
    in_maps = []
    for c in range(N_CORES):
        h, q = c // OUT_WAYS, c % OUT_WAYS
        in_maps.append({
            "xT": xT[h * tb_half : (h + 1) * tb_half],
            "nib": nib_q[q],
        })
    return in_maps


def gather_y(results, batch_shape):
    y_full = np.empty((TOKENS, OUT_F), dtype=np.float32)
    for c, r in enumerate(results):
        h, q = c // OUT_WAYS, c % OUT_WAYS
        y_full[h * TOK_C : (h + 1) * TOK_C, q * OUT_C : (q + 1) * OUT_C] = (
            unstage_y(r["y"])
        )
    return y_full.reshape(*batch_shape, OUT_F)


def kernel(x: np.ndarray, population: np.ndarray):
    in_maps = prep_inputs(x, population)
    nc = _get_nc()
    res = run_bass_kernel_spmd(nc, in_maps, core_ids=list(range(N_CORES)))
    return gather_y(res.results, x.shape[:2])
